# revision 3
# baseline (speedup 1.0000x reference)
"""Trainium2 Bass kernel for a pre-LN transformer encoder block.

Model: y = x + FFN(LN2(x + Attn(LN1(x))))  with
  D_MODEL=1024, D_FF=4096, H=16 heads, B=4, S=2048, fp32 in/out.

Sharding (8 cores, Megatron-SP): the token axis (B*S = 8192) is split 8
ways; core c owns tokens [c*1024, (c+1)*1024) (all inside batch c//2) and
a 1/8 tensor-parallel shard of every weight: heads 2c,2c+1 of wq/wk/wv
(column split), rows [c*128,(c+1)*128) of wo, d_ff slice [c*512,(c+1)*512)
of w1/w2.  LN is local to the token shard; an on-device AllGather
materializes the LN output for all 8192 tokens, each core computes its 2
heads of attention over all 4 batches (and its d_ff slice of the FFN),
and a ReduceScatter sums the partial wo/linear2 outputs back to token
shards where bias + residual are applied.

Everything big crosses the host<->device boundary in bfloat16 and exactly
once (no weight duplication), which minimizes per-call transfer: ~5MB per
core in, 2MB out.  On-device matmuls run in bf16 with fp32 PSUM
accumulation; the softmax uses the baseline's augmented-row trick (mask
row folded into K^T, ones row in Q^T, ones column on V for the
denominator) so no max-subtraction is needed.
"""

import numpy as np
import ml_dtypes

D = 1024          # d_model
DKH = 64          # head dim
AUG = 65          # head dim + 1 aug row/col
DFF_L = 512       # d_ff shard per core
R = 8             # cores
TL = 1024         # tokens per core
TG = 8192         # total tokens
P = 128
NDC = 8           # d_model chunks of 128
NEG = -1e9
EPS = 1e-5

bf16 = ml_dtypes.bfloat16
_CACHE = {}


def _build_nc():
    import concourse.bass as bass
    import concourse.tile as tile
    import concourse.mybir as mybir
    from concourse import bacc
    from concourse.bass import ts

    fp32 = mybir.dt.float32
    bf = mybir.dt.bfloat16
    AF = mybir.ActivationFunctionType
    OP = mybir.AluOpType

    nc = bacc.Bacc("TRN2", target_bir_lowering=False, debug=False, num_devices=R)

    # ---- kernel I/O (per-core shards, bf16 wire format) ----
    xsT = nc.dram_tensor("xsT", [D, TL], bf, kind="ExternalInput").ap()
    wqs = nc.dram_tensor("wqs", [D, P], bf, kind="ExternalInput").ap()
    wks = nc.dram_tensor("wks", [D, P], bf, kind="ExternalInput").ap()
    wvs = nc.dram_tensor("wvs", [D, P], bf, kind="ExternalInput").ap()
    wos = nc.dram_tensor("wos", [P, D], bf, kind="ExternalInput").ap()
    w1s = nc.dram_tensor("w1s", [D, DFF_L], bf, kind="ExternalInput").ap()
    w2s = nc.dram_tensor("w2s", [DFF_L, D], bf, kind="ExternalInput").ap()
    bqk4 = nc.dram_tensor("bqk4", [DKH, 4], fp32, kind="ExternalInput").ap()
    bvbr = nc.dram_tensor("bvbr", [1, P], fp32, kind="ExternalInput").ap()
    boc = nc.dram_tensor("boc", [P, NDC], fp32, kind="ExternalInput").ap()
    b1c = nc.dram_tensor("b1c", [P, 4], fp32, kind="ExternalInput").ap()
    b2c = nc.dram_tensor("b2c", [P, NDC], fp32, kind="ExternalInput").ap()
    madd = nc.dram_tensor("madd", [4, 2048], bf, kind="ExternalInput").ap()
    ones8k = nc.dram_tensor("ones8k", [1, TG], bf, kind="ExternalInput").ap()
    lnab = nc.dram_tensor("lnab", [1, 4], fp32, kind="ExternalInput").ap()
    yT = nc.dram_tensor("yT", [D, TL], bf, kind="ExternalOutput").ap()

    with tile.TileContext(nc) as tc:
        _emit(nc, tc, tile, mybir, ts, fp32, bf, AF, OP, locals())
    nc.compile()
    return nc


def _emit(nc, tc, tile, mybir, ts, fp32, bf, AF, OP, io):
    xsT, wqs, wks, wvs, wos = io["xsT"], io["wqs"], io["wks"], io["wvs"], io["wos"]
    w1s, w2s = io["w1s"], io["w2s"]
    bqk4, bvbr, boc, b1c, b2c = io["bqk4"], io["bvbr"], io["boc"], io["b1c"], io["b2c"]
    madd, ones8k, lnab, yT = io["madd"], io["ones8k"], io["lnab"], io["yT"]
    RG = [list(range(R))]

    from contextlib import ExitStack
    es = ExitStack()
    with es:
        es.enter_context(nc.allow_low_precision(
            reason="bf16 operands are deliberate; fp32 psum accumulation"))
        dram = es.enter_context(tc.tile_pool(name="dram", bufs=1, space="DRAM"))
        consts = es.enter_context(tc.tile_pool(name="consts", bufs=1))
        stg = es.enter_context(tc.tile_pool(name="stg", bufs=6))
        rows = es.enter_context(tc.tile_pool(name="rows", bufs=8))

        # DRAM collective bounce buffers
        ag1i = dram.tile([D, TL], bf, tag="ag1i")
        ag1o = dram.tile([TG, D], bf, tag="ag1o", addr_space="Shared")
        rs1i = dram.tile([TG, D], bf, tag="rs1i")
        rs1o = dram.tile([D, TL], bf, tag="rs1o")
        ag2i = dram.tile([D, TL], bf, tag="ag2i")
        ag2o = dram.tile([TG, D], bf, tag="ag2o", addr_space="Shared")
        rs2i = dram.tile([TG, D], bf, tag="rs2i")
        rs2o = dram.tile([D, TL], bf, tag="rs2o")

        # ---- constants ----
        bqk4_sb = consts.tile([DKH, 4], fp32, tag="bqk4")
        nc.sync.dma_start(bqk4_sb[:], bqk4[:])
        bvb_sb = consts.tile([P, P], fp32, tag="bvb")
        nc.sync.dma_start(bvb_sb[:], bvbr.to_broadcast((P, P)))
        boc_sb = consts.tile([P, NDC], fp32, tag="boc")
        nc.sync.dma_start(boc_sb[:], boc[:])
        b1c_sb = consts.tile([P, 4], fp32, tag="b1c")
        nc.sync.dma_start(b1c_sb[:], b1c[:])
        b2c_sb = consts.tile([P, NDC], fp32, tag="b2c")
        nc.sync.dma_start(b2c_sb[:], b2c[:])
        lnab_sb = consts.tile([1, 4], fp32, tag="lnab")
        nc.sync.dma_start(lnab_sb[:], lnab[:])
        ones_f = consts.tile([P, P], fp32, tag="ones_f")
        nc.vector.memset(ones_f[:], 1.0)
        ones_c = consts.tile([P, 1], bf, tag="ones_c")       # colsum lhsT
        nc.vector.tensor_copy(ones_c[:], ones_f[:, 0:1])
        ones_r = consts.tile([1, P], bf, tag="ones_r")       # bcast lhsT
        nc.vector.tensor_copy(ones_r[:], ones_f[0:1, :])

        def layer_norm_cols(x_src_fn, a_sc, b_sc, sB_ps, tB_ps, psp):
            """LN stats for one 512-token chunk; x_src_fn(c) -> [128,512] bf16.

            Fills sB_ps/tB_ps ([128,512] psum) with broadcast scale/shift:
            xn = x * sB - tB.
            """
            cx = psp.tile([1, 512], fp32, tag="sums", bufs=2)
            csq = psp.tile([1, 512], fp32, tag="sums", bufs=2)
            for c in range(NDC):
                nc.tensor.matmul(cx[:], ones_c[:], x_src_fn(c),
                                 start=(c == 0), stop=(c == NDC - 1))
            for c in range(NDC):
                sq = stg.tile([P, 512], bf, tag="stg", name="sq")
                nc.vector.tensor_mul(sq[:], x_src_fn(c), x_src_fn(c))
                nc.tensor.matmul(csq[:], ones_c[:], sq[:],
                                 start=(c == 0), stop=(c == NDC - 1))
            mean = rows.tile([1, 512], fp32, tag="rows", name="mean")
            nc.vector.tensor_scalar_mul(mean[:], cx[:], 1.0 / D)
            m2s = rows.tile([1, 512], fp32, tag="rows", name="m2s")
            nc.vector.scalar_tensor_tensor(m2s[:], mean[:], float(D) / (D - 1),
                                           mean[:], op0=OP.mult, op1=OP.mult)
            var = rows.tile([1, 512], fp32, tag="rows", name="var")
            nc.vector.scalar_tensor_tensor(var[:], csq[:], 1.0 / (D - 1),
                                           m2s[:], op0=OP.mult, op1=OP.subtract)
            std = rows.tile([1, 512], fp32, tag="rows", name="std")
            nc.scalar.activation(std[:], var[:], AF.Sqrt)
            nc.vector.tensor_scalar_add(std[:], std[:], EPS)
            rstd = rows.tile([1, 512], fp32, tag="rows", name="rstd")
            nc.vector.reciprocal(rstd[:], std[:])
            s_r = rows.tile([1, 512], bf, tag="rows", name="s_r")
            nc.vector.tensor_scalar_mul(s_r[:], rstd[:], a_sc)
            t_r = rows.tile([1, 512], bf, tag="rows", name="t_r")
            nc.vector.tensor_mul(t_r[:], mean[:], s_r[:])
            nc.vector.tensor_scalar_sub(t_r[:], t_r[:], b_sc)
            nc.tensor.matmul(sB_ps[:], ones_r[:], s_r[:], start=True, stop=True)
            nc.tensor.matmul(tB_ps[:], ones_r[:], t_r[:], start=True, stop=True)

        ag1i_r = ag1i.rearrange("(dc p) t -> p dc t", p=P)
        ag2i_r = ag2i.rearrange("(dc p) t -> p dc t", p=P)

        with tc.tile_pool(name="persistA", bufs=1) as perA:
            xsT_sb = perA.tile([P, NDC, TL], bf, tag="xsT")
            nc.sync.dma_start(xsT_sb[:], xsT.rearrange("(dc p) t -> p dc t", p=P))

            # ================= P0: LN1 -> ag1i =================
            with tc.tile_pool(name="p0", bufs=2) as p0, \
                 tc.tile_pool(name="ps0", bufs=1, space="PSUM") as ps0:
                for tci in range(2):
                    tsl = ts(tci, 512)
                    sB = ps0.tile([P, 512], fp32, tag="bcast", bufs=2)
                    tB = ps0.tile([P, 512], fp32, tag="bcast", bufs=2)
                    layer_norm_cols(lambda c: xsT_sb[:, c, tsl],
                                    lnab_sb[0:1, 0:1], lnab_sb[0:1, 1:2],
                                    sB, tB, ps0)
                    xn = p0.tile([P, NDC, 512], bf, tag="xn")
                    for c in range(NDC):
                        nc.vector.tensor_mul(xn[:, c, :], xsT_sb[:, c, tsl], sB[:])
                        nc.vector.tensor_sub(xn[:, c, :], xn[:, c, :], tB[:])
                    nc.sync.dma_start(ag1i_r[:, :, tsl], xn[:])

            nc.gpsimd.collective_compute(
                "AllGather", mybir.AluOpType.bypass, replica_groups=RG,
                ins=[ag1i.opt()], outs=[ag1o.opt()])

            # ================= P1+P2+P3: QKV, attention, wo =================
            with tc.tile_pool(name="attn_big", bufs=1) as abig:
                QT = abig.tile([AUG, 2, TG], bf, tag="QT")
                KT = abig.tile([AUG, 2, TG], bf, tag="KT")
                V_sb = abig.tile([P, 64, 2, AUG], bf, tag="V")
                CTX = abig.tile([DKH, 2, TG], bf, tag="CTX")

                nc.vector.tensor_copy(
                    V_sb[:, :, :, DKH:AUG],
                    ones_f[:, 0:1].to_broadcast((P, 64, 2, 1)))
                for h in range(2):
                    nc.sync.dma_start(QT[DKH:AUG, h, :], ones8k[0:1, :])
                    for r in range(R):
                        b = r // 2
                        s0 = (r % 2) * TL
                        nc.sync.dma_start(KT[DKH:AUG, h, r * TL:(r + 1) * TL],
                                          madd[b:b + 1, s0:s0 + TL])

                with tc.tile_pool(name="qkvw", bufs=1) as qw, \
                     tc.tile_pool(name="p1", bufs=2) as p1, \
                     tc.tile_pool(name="ps1", bufs=1, space="PSUM") as ps1:
                    wq_sb = qw.tile([P, NDC, P], bf, tag="wq")
                    nc.sync.dma_start(wq_sb[:], wqs.rearrange("(dc p) h -> p dc h", p=P))
                    wk_sb = qw.tile([P, NDC, P], bf, tag="wk")
                    nc.sync.dma_start(wk_sb[:], wks.rearrange("(dc p) h -> p dc h", p=P))
                    wv_sb = qw.tile([P, NDC, P], bf, tag="wv")
                    nc.sync.dma_start(wv_sb[:], wvs.rearrange("(dc p) h -> p dc h", p=P))

                    for r in range(R):
                        xr = p1.tile([P, NDC, TL], bf, tag="xr")
                        nc.sync.dma_start(
                            xr[:],
                            ag1o[r * TL:(r + 1) * TL, :]
                            .rearrange("(dc p) t -> p dc t", p=P))
                        for q2 in range(2):
                            qsl = ts(q2, 512)
                            goff = r * TL + q2 * 512
                            for h in range(2):
                                pq = ps1.tile([DKH, 512], fp32, tag="mmq", bufs=4)
                                for c in range(NDC):
                                    nc.tensor.matmul(
                                        pq[:], wq_sb[:, c, h * DKH:(h + 1) * DKH],
                                        xr[:, c, qsl],
                                        start=(c == 0), stop=(c == NDC - 1))
                                nc.vector.tensor_scalar_add(
                                    QT[0:DKH, h, goff:goff + 512], pq[:],
                                    bqk4_sb[:, h:h + 1])
                                pk = ps1.tile([DKH, 512], fp32, tag="mmq", bufs=4)
                                for c in range(NDC):
                                    nc.tensor.matmul(
                                        pk[:], wk_sb[:, c, h * DKH:(h + 1) * DKH],
                                        xr[:, c, qsl],
                                        start=(c == 0), stop=(c == NDC - 1))
                                nc.vector.tensor_scalar_add(
                                    KT[0:DKH, h, goff:goff + 512], pk[:],
                                    bqk4_sb[:, 2 + h:3 + h])
                        for tb in range(8):
                            pv = ps1.tile([P, P], fp32, tag="mmv", bufs=4)
                            for c in range(NDC):
                                nc.tensor.matmul(pv[:], xr[:, c, ts(tb, P)],
                                                 wv_sb[:, c, :],
                                                 start=(c == 0), stop=(c == NDC - 1))
                            blk = r * 8 + tb
                            for h in range(2):
                                nc.vector.tensor_add(
                                    V_sb[:, blk, h, 0:DKH],
                                    pv[:, h * DKH:(h + 1) * DKH],
                                    bvb_sb[:, h * DKH:(h + 1) * DKH])

                # ---- attention ----
                with tc.tile_pool(name="pr", bufs=4) as prp, \
                     tc.tile_pool(name="ps2", bufs=1, space="PSUM") as ps2:
                    for h in range(2):
                        for r in range(R):
                            b = r // 2
                            for q2 in range(2):
                                qoff = r * TL + q2 * 512
                                ctx = ps2.tile([AUG, 512], fp32, tag="ctx", bufs=2)
                                for kc2 in range(8):
                                    sc = ps2.tile([P, 2, 512], fp32, tag="sc",
                                                  bufs=2)
                                    for j in range(2):
                                        kc = kc2 * 2 + j
                                        koff = 2 * b * TL + kc * P
                                        nc.tensor.matmul(
                                            sc[:, j, :], KT[:, h, koff:koff + P],
                                            QT[:, h, qoff:qoff + 512],
                                            start=True, stop=True)
                                    pr = prp.tile([P, 2, 512], bf, tag="pr")
                                    nc.scalar.activation(pr[:], sc[:], AF.Exp,
                                                         scale=1.0 / 8.0)
                                    for j in range(2):
                                        kc = kc2 * 2 + j
                                        nc.tensor.matmul(
                                            ctx[:], V_sb[:, 16 * b + kc, h, :],
                                            pr[:, j, :],
                                            start=(kc == 0), stop=(kc == 15))
                                rr = rows.tile([1, 512], bf, tag="rows", name="rr")
                                nc.vector.reciprocal(rr[:], ctx[DKH:AUG, :])
                                rb = ps2.tile([DKH, 512], fp32, tag="rb", bufs=2)
                                nc.tensor.matmul(rb[:], ones_r[0:1, 0:DKH], rr[:],
                                                 start=True, stop=True)
                                cst = stg.tile([DKH, 512], bf, tag="stg",
                                               name="cst")
                                nc.vector.tensor_copy(cst[:], ctx[0:DKH, :])
                                nc.vector.tensor_mul(CTX[:, h, qoff:qoff + 512],
                                                     cst[:], rb[:])

                # ---- wo partial -> rs1i ----
                with tc.tile_pool(name="wop", bufs=1) as wop, \
                     tc.tile_pool(name="ps3", bufs=1, space="PSUM") as ps3:
                    wos_sb = wop.tile([DKH, 2, D], bf, tag="wos")
                    nc.sync.dma_start(wos_sb[:], wos.rearrange("(h e) o -> e h o", h=2))
                    for r in range(R):
                        for q2 in range(2):
                            qoff = r * TL + q2 * 512
                            for do in range(NDC):
                                po = ps3.tile([P, 512], fp32, tag="mm", bufs=4)
                                for h in range(2):
                                    nc.tensor.matmul(
                                        po[:], wos_sb[:, h, ts(do, P)],
                                        CTX[:, h, qoff:qoff + 512],
                                        start=(h == 0), stop=(h == 1))
                                st = stg.tile([P, 512], bf, tag="stg", name="po")
                                nc.vector.tensor_copy(st[:], po[:])
                                nc.sync.dma_start(
                                    rs1i[r * TL + do * P:r * TL + (do + 1) * P,
                                         ts(q2, 512)],
                                    st[:])

            nc.gpsimd.collective_compute(
                "ReduceScatter", mybir.AluOpType.add, replica_groups=RG,
                ins=[rs1i.opt()], outs=[rs1o.opt()])

            # ================= P4: residual + LN2 -> ag2i =================
            with tc.tile_pool(name="persistB", bufs=1) as perB:
                x2 = perB.tile([P, NDC, TL], bf, tag="x2")
                with tc.tile_pool(name="p4", bufs=1) as p4, \
                     tc.tile_pool(name="ps4", bufs=1, space="PSUM") as ps4:
                    rs_sb = p4.tile([P, NDC, TL], bf, tag="rs")
                    nc.sync.dma_start(rs_sb[:],
                                      rs1o.rearrange("(dc p) t -> p dc t", p=P))
                    for c in range(NDC):
                        nc.vector.scalar_tensor_tensor(
                            x2[:, c, :], rs_sb[:, c, :], boc_sb[:, c:c + 1],
                            xsT_sb[:, c, :], op0=OP.add, op1=OP.add)
                    for tci in range(2):
                        tsl = ts(tci, 512)
                        sB = ps4.tile([P, 512], fp32, tag="bcast", bufs=2)
                        tB = ps4.tile([P, 512], fp32, tag="bcast", bufs=2)
                        layer_norm_cols(lambda c: x2[:, c, tsl],
                                        lnab_sb[0:1, 2:3], lnab_sb[0:1, 3:4],
                                        sB, tB, ps4)
                        xn = p4.tile([P, NDC, 512], bf, tag="xn2", bufs=2)
                        for c in range(NDC):
                            nc.vector.tensor_mul(xn[:, c, :], x2[:, c, tsl], sB[:])
                            nc.vector.tensor_sub(xn[:, c, :], xn[:, c, :], tB[:])
                        nc.sync.dma_start(ag2i_r[:, :, tsl], xn[:])

                nc.gpsimd.collective_compute(
                    "AllGather", mybir.AluOpType.bypass, replica_groups=RG,
                    ins=[ag2i.opt()], outs=[ag2o.opt()])

                # ================= P5+P6: FFN -> rs2i =================
                with tc.tile_pool(name="ffn", bufs=1) as ffn, \
                     tc.tile_pool(name="p5", bufs=2) as p5, \
                     tc.tile_pool(name="ps5", bufs=1, space="PSUM") as ps5:
                    w1_sb = ffn.tile([P, NDC, DFF_L], bf, tag="w1")
                    nc.sync.dma_start(w1_sb[:],
                                      w1s.rearrange("(dc p) f -> p dc f", p=P))
                    w2_sb = ffn.tile([P, 4, D], bf, tag="w2")
                    nc.sync.dma_start(w2_sb[:],
                                      w2s.rearrange("(j p) o -> p j o", p=P))
                    H1 = ffn.tile([P, 4, TG], bf, tag="H1")
                    for r in range(R):
                        xr = p5.tile([P, NDC, TL], bf, tag="xr2")
                        nc.sync.dma_start(
                            xr[:],
                            ag2o[r * TL:(r + 1) * TL, :]
                            .rearrange("(dc p) t -> p dc t", p=P))
                        for f in range(4):
                            for q2 in range(2):
                                hp = ps5.tile([P, 512], fp32, tag="h1m", bufs=4)
                                for c in range(NDC):
                                    nc.tensor.matmul(
                                        hp[:], w1_sb[:, c, ts(f, P)],
                                        xr[:, c, ts(q2, 512)],
                                        start=(c == 0), stop=(c == NDC - 1))
                                nc.vector.tensor_scalar(
                                    H1[:, f, r * TL + q2 * 512:
                                       r * TL + q2 * 512 + 512],
                                    hp[:], b1c_sb[:, f:f + 1], 0.0,
                                    op0=OP.add, op1=OP.max)
                    for r in range(R):
                        for q2 in range(2):
                            goff = r * TL + q2 * 512
                            for do in range(NDC):
                                h2p = ps5.tile([P, 512], fp32, tag="h2m", bufs=4)
                                for j in range(4):
                                    nc.tensor.matmul(
                                        h2p[:], w2_sb[:, j, ts(do, P)],
                                        H1[:, j, goff:goff + 512],
                                        start=(j == 0), stop=(j == 3))
                                st = stg.tile([P, 512], bf, tag="stg", name="h2")
                                nc.vector.tensor_copy(st[:], h2p[:])
                                nc.sync.dma_start(
                                    rs2i[r * TL + do * P:r * TL + (do + 1) * P,
                                         ts(q2, 512)],
                                    st[:])

                nc.gpsimd.collective_compute(
                    "ReduceScatter", mybir.AluOpType.add, replica_groups=RG,
                    ins=[rs2i.opt()], outs=[rs2o.opt()])

                # ================= P7: bias + residual -> yT =================
                with tc.tile_pool(name="p7", bufs=2) as p7:
                    rs2_sb = p7.tile([P, NDC, TL], bf, tag="rs2")
                    nc.sync.dma_start(rs2_sb[:],
                                      rs2o.rearrange("(dc p) t -> p dc t", p=P))
                    for c in range(NDC):
                        yst = p7.tile([P, TL], bf, tag="yst", bufs=4)
                        nc.vector.scalar_tensor_tensor(
                            yst[:], rs2_sb[:, c, :], b2c_sb[:, c:c + 1],
                            x2[:, c, :], op0=OP.add, op1=OP.add)
                        nc.sync.dma_start(yT[c * P:(c + 1) * P, :], yst[:])


def _get_nc():
    if "nc" not in _CACHE:
        _CACHE["nc"] = _build_nc()
    return _CACHE["nc"]


def _make_in_maps(x, src_mask, wq, bq, wk, bk, wv, bv, wo, bo,
                  w1, b1, w2, b2, ln1_a, ln1_b, ln2_a, ln2_b):
    f = np.float32
    xb = np.asarray(x, f).reshape(TG, D).astype(bf16)
    wqb = np.asarray(wq, f).astype(bf16)
    wkb = np.asarray(wk, f).astype(bf16)
    wvb = np.asarray(wv, f).astype(bf16)
    wob = np.asarray(wo, f).astype(bf16)
    w1b = np.asarray(w1, f).astype(bf16)
    w2b = np.asarray(w2, f).astype(bf16)
    bqf = np.asarray(bq, f); bkf = np.asarray(bk, f); bvf = np.asarray(bv, f)
    mrow = np.where(np.asarray(src_mask).reshape(4, 2048) == 0,
                    f(8.0 * NEG), f(0.0)).astype(bf16)
    common = {
        "madd": mrow,
        "ones8k": np.ones((1, TG), bf16),
        "boc": np.ascontiguousarray(np.asarray(bo, f).reshape(NDC, P).T),
        "b2c": np.ascontiguousarray(np.asarray(b2, f).reshape(NDC, P).T),
        "lnab": np.array([[np.asarray(ln1_a).reshape(-1)[0],
                           np.asarray(ln1_b).reshape(-1)[0],
                           np.asarray(ln2_a).reshape(-1)[0],
                           np.asarray(ln2_b).reshape(-1)[0]]], f),
    }
    in_maps = []
    for c in range(R):
        m = dict(common)
        m["xsT"] = np.ascontiguousarray(xb[c * TL:(c + 1) * TL, :].T)
        m["wqs"] = np.ascontiguousarray(wqb[:, c * P:(c + 1) * P])
        m["wks"] = np.ascontiguousarray(wkb[:, c * P:(c + 1) * P])
        m["wvs"] = np.ascontiguousarray(wvb[:, c * P:(c + 1) * P])
        m["wos"] = np.ascontiguousarray(wob[c * P:(c + 1) * P, :])
        m["w1s"] = np.ascontiguousarray(w1b[:, c * DFF_L:(c + 1) * DFF_L])
        m["w2s"] = np.ascontiguousarray(w2b[c * DFF_L:(c + 1) * DFF_L, :])
        m["bqk4"] = np.ascontiguousarray(np.stack(
            [bqf[c * P:c * P + DKH], bqf[c * P + DKH:(c + 1) * P],
             bkf[c * P:c * P + DKH], bkf[c * P + DKH:(c + 1) * P]], axis=1))
        m["bvbr"] = np.ascontiguousarray(bvf[c * P:(c + 1) * P].reshape(1, P))
        m["b1c"] = np.ascontiguousarray(
            np.asarray(b1, f)[c * DFF_L:(c + 1) * DFF_L].reshape(4, P).T)
        in_maps.append(m)
    return in_maps


def kernel(**inputs):
    from concourse import bass_utils

    nc = _get_nc()
    in_maps = _make_in_maps(**inputs)
    res = bass_utils.run_bass_kernel_spmd(nc, in_maps, core_ids=list(range(R)))
    out = np.empty((TG, D), np.float32)
    for c in range(R):
        out[c * TL:(c + 1) * TL, :] = res.results[c]["yT"].T
    return out.reshape(4, 2048, D)
